# revision 19
# baseline (speedup 1.0000x reference)
"""NT-Xent / SimCLR contrastive loss on 8 Trainium2 NeuronCores.

Problem: emb_i, emb_j [4096, 1024] f32 -> scalar loss.
  z = l2norm(rows); reps = concat(z_i, z_j) [2N, D]
  sim = reps @ reps.T;  loss = mean(-(pos/T - log(sum_offdiag exp(sim/T))))

Sharding (data parallel over the 2N=8192 rows, 1024 rows per core):
  - each core normalizes its 1024 local rows (scaled by S=64 so values sit
    in the fp8-e4m3 normal range), transposes them to [D, rows] fp8, and
    AllGathers the transposed fp8 matrix (1 MiB per core),
  - computes its [1024, 8192] sim block with TensorE fp8 DoubleRow
    matmuls (2x contraction per pass, f32 accum), fusing
    exp(2*sim) = exp((2/S^2) * psum) + row-sum into one wide [128,1024]
    ScalarE activation per (block, m) pair,
  - positives are computed by a separate f32 path (host supplies each
    core's partner row block), which keeps the single SPMD program free
    of core-dependent addressing; the self-similarity diagonal term is
    exp(2) to within fp8 rounding and is subtracted as a constant,
  - per-row partial losses [128, 8] go back to the host, which sums and
    scales: a trivial gather.

Host-side work is only sharding/assembly: slicing rows, one np.eye, and a
final sum of the 8192 per-row loss terms.
"""

import math

import numpy as np
import ml_dtypes

import concourse.bacc as bacc
import concourse.bass as bass
import concourse.mybir as mybir
import concourse.tile as tile
from concourse.bass_utils import run_bass_kernel_spmd

FP32 = mybir.dt.float32
BF16 = mybir.dt.bfloat16
FP8 = mybir.dt.float8e4
AF = mybir.ActivationFunctionType
ALU = mybir.AluOpType
PM = mybir.MatmulPerfMode

C = 8         # cores
N = 4096      # batch (per view)
D = 1024      # embedding dim
R = 1024      # local rows per core (2N / C)
P = 128       # partitions
MT = R // P   # m-tiles per core (8)
NT = 512      # PSUM bank free size (f32)
ESCALE = 2.0  # 1 / temperature
S = 64.0      # fp8 pre-quantization scale; exp scale folds in 1/S^2
LNS = math.log(S)
QSCALE = ESCALE / (S * S)
EDIAG = math.exp(ESCALE)  # self-sim diagonal term, exact to fp8 rounding


def _build_kernel(tc, nc, xloc, xpart, ident, out):
    with (
        tc.tile_pool(name="constp", bufs=1) as constp,
        tc.tile_pool(name="xmp", bufs=1) as xmp,      # 8 persistent local f32 tiles
        tc.tile_pool(name="pmp", bufs=1) as pmp,      # 8 persistent partner f32 tiles
        tc.tile_pool(name="zmp", bufs=1) as zmp,      # 8 persistent fp8 z tiles
        tc.tile_pool(name="ztp", bufs=1) as ztp,      # one [P, MT, R] fp8 zT tile
        tc.tile_pool(name="statp", bufs=1) as statp,
        tc.tile_pool(name="scrp", bufs=3) as scrp,    # [P, D] discard scratch
        tc.tile_pool(name="gp", bufs=2) as gp,        # gathered fp8 tiles
        tc.tile_pool(name="psp", bufs=3, space="PSUM") as psp,   # [P,2*NT] = 2 banks each
        tc.tile_pool(name="ptp", bufs=2, space="PSUM") as ptp,   # fp8 transpose staging
        tc.tile_pool(name="expp", bufs=4) as expp,
        tc.tile_pool(name="raccp", bufs=1) as raccp,
        tc.tile_pool(name="dramp", bufs=1, space="DRAM") as dramp,
    ):
        identt = constp.tile([P, P], BF16, name="identt")
        nc.sync.dma_start(identt[:], ident[:])

        lns = statp.tile([P, 1], FP32, name="lns")
        nc.vector.memset(lns[:], LNS)

        ss = statp.tile([P, MT], FP32, name="ss")
        ssp = statp.tile([P, MT], FP32, name="ssp")
        upos = statp.tile([P, MT], FP32, name="upos")
        rs = statp.tile([P, MT], FP32, name="rs")
        lss = statp.tile([P, MT], FP32, name="lss")

        # ---- phase 1: local+partner row norms, scale, transpose to fp8 ----
        # All ScalarE Squares are grouped (one act-table load), then a
        # single Ln+Exp pair computes rs = S/||x|| = exp(-.5*ln(ss)+ln(S))
        # for all 8 row tiles at once (Rsqrt ACT is banned).
        xms = []
        pms = []
        for m in range(MT):
            xm = xmp.tile([P, D], FP32, name=f"xm{m}", tag=f"xm{m}")
            nc.sync.dma_start(xm[:], xloc[m * P:(m + 1) * P, :])
            sq = scrp.tile([P, D], FP8, name="sq", tag="scr")
            nc.scalar.activation(sq[:], xm[:], AF.Square,
                                 accum_out=ss[:, m:m + 1])
            xms.append(xm)
        nc.scalar.activation(lss[:], ss[:], AF.Ln)
        nc.scalar.activation(rs[:], lss[:], AF.Exp, scale=-0.5, bias=lns[:])
        # partner rows: DMAs must land pre-AG (quiesce), squares run post-AG
        pm_dmas = []
        for m in range(MT):
            pm = pmp.tile([P, D], FP32, name=f"pm{m}", tag=f"pm{m}")
            pm_dmas.append(nc.sync.dma_start(pm[:], xpart[m * P:(m + 1) * P, :]))
            pms.append(pm)

        zt = ztp.tile([P, MT, R], FP8, name="zt")
        for m in range(MT):
            zm = zmp.tile([P, D], BF16, name=f"zm{m}", tag=f"zm{m}")
            nc.vector.tensor_scalar_mul(zm[:], xms[m][:], rs[:, m:m + 1])
            pt = ptp.tile([P, MT, P], BF16, name="pt", tag="pt")
            for d in range(8):
                nc.tensor.matmul(pt[:, d, :], zm[:, d * P:(d + 1) * P],
                                 identt[:], is_transpose=True,
                                 skip_group_check=True)
            # cast-copy on ScalarE (Copy needs no act table); DVE keeps the muls
            nc.scalar.activation(zt[:, :, m * P:(m + 1) * P], pt[:], AF.Copy)

        # ---- phase 2: AllGather the normalized transposed fp8 reps ----
        # The collective must run quiesced: concurrent DMA/engine activity
        # during a collective wedges this terminal's NRT (hang /
        # NRT_EXEC_UNIT_UNRECOVERABLE). Hence the explicit fences below.
        # NOTE: addr_space="Shared" outputs >~2 MiB wedge this terminal's
        # NRT (NRT_EXEC_UNIT_UNRECOVERABLE); Local outputs work at 16 MiB.
        ag_in = dramp.tile([R, R], FP8, name="ag_in")
        ag_out = dramp.tile([C * R, R], FP8, name="ag_out")
        asm_dmas = []
        for d in range(8):
            asm_dmas.append(
                nc.sync.dma_start(ag_in[d * P:(d + 1) * P, :], zt[:, d, :]))
        cc = nc.gpsimd.collective_compute(
            "AllGather",
            ALU.bypass,
            replica_groups=[list(range(C))],
            ins=[ag_in[:].opt()],
            outs=[ag_out[:].opt()],
        )
        # Quiesce: no in-flight phase-1 DMAs while a collective runs.
        for dma in asm_dmas + pm_dmas:
            tile.add_dep_helper(cc.ins, dma.ins,
                                reason="collective after all phase-1 DMAs")
        last_cc = cc

        # ---- phase 1b: positives (after the collective — nothing may
        # overlap the AG window, see above; runs on DVE while TensorE
        # streams phase-3 matmuls; the ScalarE part is spliced into the
        # middle of phase 3 where ScalarE has slack, to keep the early
        # phase-3 exps from stalling the matmul pipeline) ----
        for m in range(MT):
            # (tensor_tensor_reduce faults this terminal's NRT with an
            # INTERNAL error — use plain mul + reduce instead)
            um = scrp.tile([P, D], FP32, name="um", tag="scr")
            umi = nc.vector.tensor_mul(um[:], xms[m][:], pms[m][:])
            tile.add_dep_helper(umi.ins, last_cc.ins,
                                reason="no engine work during collectives")
            nc.vector.reduce_sum(upos[:, m:m + 1], um[:],
                                 axis=mybir.AxisListType.X)

        # partner norms on the idle DVE (ScalarE is busy with phase-3 exps)
        for m in range(MT):
            sqp = scrp.tile([P, D], FP32, name="sqp", tag="scr")
            sqi = nc.vector.tensor_mul(sqp[:], pms[m][:], pms[m][:])
            tile.add_dep_helper(sqi.ins, last_cc.ins,
                                reason="no engine work during collectives")
            nc.vector.reduce_sum(ssp[:, m:m + 1], sqp[:],
                                 axis=mybir.AxisListType.X)

        def positives_tail():
            # rsp + pos2, issued mid-phase-3 (small ScalarE lump fits slack)
            lssp = statp.tile([P, MT], FP32, name="lssp")
            nc.scalar.activation(lssp[:], ssp[:], AF.Ln)
            rsp = statp.tile([P, MT], FP32, name="rsp")
            nc.scalar.activation(rsp[:], lssp[:], AF.Exp, scale=-0.5,
                                 bias=lns[:])
            # pos2 = 2 * upos * (S/||x||) * (S/||p||) / S^2
            t1 = statp.tile([P, MT], FP32, name="t1")
            nc.vector.tensor_mul(t1[:], upos[:], rs[:])
            t2 = statp.tile([P, MT], FP32, name="t2")
            nc.vector.tensor_mul(t2[:], t1[:], rsp[:])
            pos2 = statp.tile([P, MT], FP32, name="pos2")
            nc.vector.tensor_scalar_mul(pos2[:], t2[:], QSCALE)
            return pos2

        # ---- phase 3: sim blocks + fused exp/row-sum ----
        # fp8 DoubleRow: each matmul consumes two 128-deep k-chunks.
        racc = raccp.tile([P, MT, C], FP32, name="racc")
        pos2 = None
        for cb in range(C):
            if cb == 5:
                pos2 = positives_tail()
            g = gp.tile([P, MT, R], FP8, name="g", tag="g")
            for d in range(8):
                gd = nc.sync.dma_start(
                    g[:, d, :],
                    ag_out[(cb * 8 + d) * P:(cb * 8 + d + 1) * P, :])
                tile.add_dep_helper(gd.ins, last_cc.ins,
                                    reason="no DMA during collectives")
            for m in range(MT):
                ps = psp.tile([P, 2 * NT], FP32, name="ps", tag="ps")
                for k in range(4):
                    lhs = zt[:, 2 * k:2 * k + 2, m * P:(m + 1) * P]
                    nc.tensor.matmul(ps[:, 0:NT], lhs,
                                     g[:, 2 * k:2 * k + 2, 0:NT],
                                     start=(k == 0), stop=(k == 3),
                                     perf_mode=PM.DoubleRow)
                    nc.tensor.matmul(ps[:, NT:2 * NT], lhs,
                                     g[:, 2 * k:2 * k + 2, NT:2 * NT],
                                     start=(k == 0), stop=(k == 3),
                                     perf_mode=PM.DoubleRow)
                ed = expp.tile([P, 2 * NT], FP8, name="ed", tag="ed")
                nc.scalar.activation(ed[:], ps[:], AF.Exp, scale=QSCALE,
                                     accum_out=racc[:, m, cb:cb + 1])

        # ---- tail: denom, log, per-row loss ----
        rstot = statp.tile([P, MT], FP32, name="rstot")
        nc.vector.reduce_sum(rstot[:], racc[:], axis=mybir.AxisListType.X)
        denom = statp.tile([P, MT], FP32, name="denom")
        nc.vector.tensor_scalar_sub(denom[:], rstot[:], EDIAG)
        logd = statp.tile([P, MT], FP32, name="logd")
        nc.scalar.activation(logd[:], denom[:], AF.Ln)
        outv = statp.tile([P, MT], FP32, name="outv")
        nc.vector.tensor_sub(outv[:], logd[:], pos2[:])
        nc.sync.dma_start(out[:], outv[:])


_NC_CACHE = {}


def build_nc():
    if "nc" in _NC_CACHE:
        return _NC_CACHE["nc"]
    nc = bacc.Bacc("TRN2", target_bir_lowering=False, debug=False,
                   num_devices=C)
    xloc = nc.dram_tensor("xloc", [R, D], FP32, kind="ExternalInput")
    xpart = nc.dram_tensor("xpart", [R, D], FP32, kind="ExternalInput")
    ident = nc.dram_tensor("ident", [P, P], BF16, kind="ExternalInput")
    out = nc.dram_tensor("out", [P, MT], FP32, kind="ExternalOutput")
    with tile.TileContext(nc) as tc:
        _build_kernel(tc, nc, xloc, xpart, ident, out)
    nc.compile()
    _NC_CACHE["nc"] = nc
    return nc


def run(emb_i, emb_j, **spmd_kwargs):
    x = np.concatenate(
        [np.asarray(emb_i, dtype=np.float32),
         np.asarray(emb_j, dtype=np.float32)], axis=0)
    eye = np.eye(P, dtype=ml_dtypes.bfloat16)
    in_maps = []
    for c in range(C):
        p = (c + C // 2) % C
        in_maps.append({
            "xloc": np.ascontiguousarray(x[c * R:(c + 1) * R]),
            "xpart": np.ascontiguousarray(x[p * R:(p + 1) * R]),
            "ident": eye,
        })
    nc = build_nc()
    res = run_bass_kernel_spmd(nc, in_maps, core_ids=list(range(C)),
                               **spmd_kwargs)
    total = np.float64(0.0)
    for c in range(C):
        total += np.asarray(res.results[c]["out"], dtype=np.float64).sum()
    loss = np.float32(total / (2 * N))
    return loss, res


def kernel(emb_i, emb_j):
    loss, _ = run(emb_i, emb_j)
    return np.asarray(loss, dtype=np.float32)
